# revision 10
# baseline (speedup 1.0000x reference)
"""Trainium2 Bass kernel for nn_Net_77841987273494 (GNN message passing).

Strategy (8 NeuronCores, dst-sharded):
  - Nodes are partitioned into 8 contiguous shards by destination.
  - GCN normalization is factored:  out[d] = dinv[d] * sum_{e->d} w_e * g[src_e]
    with g = dinv ⊙ (X @ W) (row-scaled feature table), so per-edge work is a
    row gather + a weighted one-hot matmul reduction on the PE:
        acc[f, d] += M[e, f]^T @ onehot[e, d],  onehot = (iota==dstloc)*w
    built in one fused DVE tensor_scalar per 128-edge group.
  - Conv1 table g1 is computed redundantly per core from host-staged
    xsT = (dinv ⊙ x)^T; conv2 table g2 is computed shard-wise and AllGathered.
  - Edge row gathers use the GPSIMD dma_gather custom instruction; int16
    indices require splitting the table at row 32768 (two passes per conv).
  - Readout: scores via PE, exact local top-16 extraction, small AllGather of
    mean-pool partials + top-k candidates, global merge + emb on every core,
    chosen rows fetched by the owning shard via dma_gather.

The harness calls kernel(**inputs) with full-size numpy inputs and gets the
full [1, 2816] output back.
"""

import math

import numpy as np

import concourse.bacc as bacc
import concourse.bass as bass
import concourse.mybir as mybir
import concourse.tile as tile
from concourse.bass_utils import run_bass_kernel_spmd

F32 = mybir.dt.float32
I16 = mybir.dt.int16
U32 = mybir.dt.uint32
AF = mybir.ActivationFunctionType
OP = mybir.AluOpType
AX = mybir.AxisListType

P = 128
N_CORES = 8
CHUNK_G = 32          # groups (of 128 edges) per dma_gather instruction
NEG = -1.0e30


# ----------------------------------------------------------------------------
# Host-side graph preprocessing
# ----------------------------------------------------------------------------

def _preprocess(x, attributes, edge_weight, edge_index, hidden_state,
                W1, b1, W2, b2, Wmap, bmap, Wattn):
    x = np.asarray(x, np.float32)
    attributes = np.asarray(attributes, np.float32)
    N, DIN = x.shape
    DATTR = attributes.shape[1]
    shard = N // N_CORES
    assert shard * N_CORES == N
    T = math.ceil(shard / P)                      # dst tiles per core
    last_rows = shard - (T - 1) * P               # rows in last tile
    split = 32768 if N > 32768 else ((N // 2 + P - 1) // P) * P
    ntab = ((N + P - 1) // P) * P                 # padded table rows

    src = np.asarray(edge_index[0], dtype=np.int64)
    dst = np.asarray(edge_index[1], dtype=np.int64)
    w = np.asarray(edge_weight, dtype=np.float64)

    # self loops (A + I)
    loop = np.arange(N, dtype=np.int64)
    src2 = np.concatenate([src, loop])
    dst2 = np.concatenate([dst, loop])
    w2 = np.concatenate([w, np.ones(N)])

    deg = np.bincount(dst2, weights=w2, minlength=N)
    dinv = np.where(deg > 0, 1.0 / np.sqrt(np.maximum(deg, 1e-12)), 0.0)
    dinv32 = dinv.astype(np.float32)

    core = dst2 // shard
    t_of = (dst2 - core * shard) // P
    half = (src2 >= split).astype(np.int64)
    dloc = dst2 - core * shard - t_of * P

    # order edges by (core, half, tile, src)
    order = np.lexsort((src2, t_of, half, core))
    core_s, half_s, t_s = core[order], half[order], t_of[order]
    src_s, dloc_s, w_s = src2[order], dloc[order], w2[order]

    # unified bucket sizes: per (half, tile) padded max count over cores
    key = (core_s * 2 + half_s) * T + t_s
    cnt_all = np.bincount(key, minlength=N_CORES * 2 * T)
    cnt = cnt_all.reshape(N_CORES, 2 * T)
    bucket_groups = ((cnt.max(axis=0) + P - 1) // P).reshape(2, T)
    gstart = np.zeros((2, T), dtype=np.int64)
    acc = 0
    for h in range(2):
        for t in range(T):
            gstart[h, t] = acc
            acc += bucket_groups[h, t]
    G = int(acc)

    # gather chunks (cannot straddle the half boundary)
    g_pass0 = int(bucket_groups[0].sum())
    chunks = []           # (half, g0, ng)
    for h, lo, hi in ((0, 0, g_pass0), (1, g_pass0, G)):
        g = lo
        while g < hi:
            ng = min(CHUNK_G, hi - g)
            chunks.append((h, g, ng))
            g += ng

    group_tile = np.zeros(max(G, 1), dtype=np.int64)
    for h in range(2):
        for t in range(T):
            g0, ng = gstart[h, t], bucket_groups[h, t]
            group_tile[g0:g0 + ng] = t

    # per-core padded edge arrays
    idx_cols = G * P // 16
    starts_sorted = np.concatenate([[0], np.cumsum(cnt_all)[:-1]])
    rank = np.arange(len(order)) - np.repeat(starts_sorted, cnt_all)
    per_core = []
    for c in range(N_CORES):
        sel = core_s == c
        h_c, t_c = half_s[sel], t_s[sel]
        src_c, dloc_c, w_c, rank_c = src_s[sel], dloc_s[sel], w_s[sel], rank[sel]
        pos = gstart[h_c, t_c] * P + rank_c
        srcpad = np.zeros(G * P, dtype=np.int64)
        wpad = np.zeros(G * P, dtype=np.float32)
        dstpad = np.zeros(G * P, dtype=np.float32)
        srcpad[pos] = np.where(h_c == 1, src_c - split, src_c)
        wpad[pos] = (w_c * dinv[dst2[order][sel]]).astype(np.float32)
        dstpad[pos] = dloc_c.astype(np.float32)

        idx_all = np.zeros((16, idx_cols), dtype=np.int16)
        for (h, g0, ng) in chunks:
            blk = srcpad[g0 * P:(g0 + ng) * P].astype(np.int16)
            idx_all[:, g0 * 8:(g0 + ng) * 8] = blk.reshape(ng * 8, 16).T
        idx_all = np.tile(idx_all, (8, 1))                 # [128, idx_cols]
        per_core.append(dict(
            idx_all=idx_all,
            dstloc_all=dstpad.reshape(G, P).T.copy(),
            w_all=wpad.reshape(G, P).T.copy()))

    # dense host-staged tensors (shared across cores)
    xs = dinv32[:, None] * x
    xsT = np.zeros((DIN, ntab), dtype=np.float32)
    xsT[:, :N] = xs.T
    DH = np.asarray(hidden_state).shape[1]
    KH = DH // P
    DMID = W1.shape[1]
    DOUT = W2.shape[1]
    DH3 = DOUT + DATTR
    hsT_tiles = np.asarray(hidden_state, np.float32).reshape(KH, P).T.copy()
    Wattn_flat = (np.asarray(Wattn, np.float32).reshape(KH, P, DH3)
                  .transpose(1, 0, 2).reshape(P, KH * DH3).copy())

    meta = dict(N=N, DIN=DIN, DATTR=DATTR, shard=shard, T=T,
                last_rows=last_rows, split=split, ntab=ntab, G=G,
                g_pass0=g_pass0, chunks=chunks, group_tile=group_tile,
                gstart=gstart, bucket_groups=bucket_groups,
                idx_cols=idx_cols, KH=KH, DMID=DMID, DOUT=DOUT,
                DEMB=Wmap.shape[1], K=16)

    shared = dict(xsT=xsT,
                  iota_t=np.tile(np.arange(P, dtype=np.float32), (P, 1)),
                  ident=np.eye(P, dtype=np.float32),
                  W1=np.asarray(W1, np.float32),
                  W2a=np.asarray(W2[:DMID], np.float32),
                  W2b=np.asarray(W2[DMID:], np.float32),
                  b1col=np.asarray(b1, np.float32).reshape(-1, 1),
                  b2col=np.asarray(b2, np.float32).reshape(-1, 1),
                  Wmap_lo=np.asarray(Wmap[:P], np.float32),
                  Wmap_hi=np.asarray(Wmap[P:], np.float32),
                  bmap=np.asarray(bmap, np.float32).reshape(1, -1),
                  hsT_tiles=hsT_tiles, Wattn_flat=Wattn_flat,
                  ones8=np.ones((8, 1), np.float32),
                  ones_1x128=np.ones((1, P), np.float32))

    for c in range(N_CORES):
        d = per_core[c]
        base = c * shard
        att = attributes[base:base + shard]
        att_tiles = np.zeros((P, T, DATTR), dtype=np.float32)
        for t in range(T):
            r = min(P, shard - t * P)
            att_tiles[:r, t, :] = att[t * P:t * P + r]
        attT = np.zeros((DATTR, T * P), dtype=np.float32)
        attT[:, :shard] = att.T
        dinv_sh = dinv32[base:base + shard]
        tmp = np.zeros(P * T, dtype=np.float32)
        tmp[:shard] = dinv_sh
        dinv_col = tmp.reshape(T, P).T.copy()
        idx_grid = np.full((P, 56), -1.0, dtype=np.float32)
        for t in range(T):
            r = min(P, shard - t * P)
            idx_grid[:r, t] = base + t * P + np.arange(r)
        d.update(att_tiles=att_tiles.reshape(P, T * DATTR), attT=attT,
                 dinv_col=dinv_col, idx_grid=idx_grid,
                 pcol_base=(np.arange(P, dtype=np.float32) + base).reshape(P, 1),
                 base_11=np.full((1, 1), float(base), np.float32),
                 invN=np.full((P, 1), 1.0 / N, np.float32))
    return meta, shared, per_core


# ----------------------------------------------------------------------------
# Device program
# ----------------------------------------------------------------------------

def build_program(meta, debug=False):
    N, T, G = meta["N"], meta["T"], meta["G"]
    shard, split, ntab = meta["shard"], meta["split"], meta["ntab"]
    DIN, DMID, DOUT, DATTR = meta["DIN"], meta["DMID"], meta["DOUT"], meta["DATTR"]
    DEMB, KH, K = meta["DEMB"], meta["KH"], meta["K"]
    DH3 = DOUT + DATTR                       # 160
    DH3P = ((DH3 * 4 + 255) // 256) * 64     # padded h3 row in f32 elems (192)
    chunks, group_tile = meta["chunks"], meta["group_tile"]
    gstart, bucket_groups = meta["gstart"], meta["bucket_groups"]
    idx_cols = meta["idx_cols"]
    n_table_tiles = ntab // P
    last_rows = meta["last_rows"]

    nc = bacc.Bacc("TRN2", target_bir_lowering=False, debug=False,
                   enable_asserts=False, num_devices=N_CORES)

    def din(name, shape, dtype=F32):
        return nc.dram_tensor(name, shape, dtype, kind="ExternalInput")

    xsT = din("xsT", [DIN, ntab])
    iota_t = din("iota_t", [P, P])
    ident = din("ident", [P, P])
    W1 = din("W1", [DIN, DMID])
    W2a = din("W2a", [DMID, DOUT])
    W2b = din("W2b", [DATTR, DOUT])
    b1col = din("b1col", [DMID, 1])
    b2col = din("b2col", [DOUT, 1])
    Wmap_lo = din("Wmap_lo", [P, DEMB])
    Wmap_hi = din("Wmap_hi", [DH3 - P, DEMB])
    bmap = din("bmap", [1, DEMB])
    hsT_tiles = din("hsT_tiles", [P, KH])
    Wattn_flat = din("Wattn_flat", [P, KH * DH3])
    ones8 = din("ones8", [8, 1])
    ones_1x128 = din("ones_1x128", [1, P])
    idx_all = din("idx_all", [P, idx_cols], I16)
    dstloc_all = din("dstloc_all", [P, G])
    w_all = din("w_all", [P, G])
    att_tiles = din("att_tiles", [P, T * DATTR])
    attT = din("attT", [DATTR, T * P])
    dinv_col = din("dinv_col", [P, T])
    idx_grid = din("idx_grid", [P, 56])
    pcol_base = din("pcol_base", [P, 1])
    base_11 = din("base_11", [1, 1])
    invN = din("invN", [P, 1])

    emb_out = nc.dram_tensor("emb_out", [1, DEMB], F32, kind="ExternalOutput")
    if debug:
        zT_out = nc.dram_tensor("zT_out", [P, T * P], F32, kind="ExternalOutput")
        o2T_out = nc.dram_tensor("o2T_out", [P, T * P], F32, kind="ExternalOutput")
        sc_out = nc.dram_tensor("sc_out", [P, 56], F32, kind="ExternalOutput")
        g1t_out = nc.dram_tensor("g1t_out", [256, DMID], F32, kind="ExternalOutput")
        g2t_out = nc.dram_tensor("g2t_out", [256, DOUT], F32, kind="ExternalOutput")
        ti_out = nc.dram_tensor("ti_out", [1, K], F32, kind="ExternalOutput")
        tig_out = nc.dram_tensor("tig_out", [1, K], F32, kind="ExternalOutput")
    chose_out = nc.dram_tensor("chose_out", [K, DH3], F32, kind="ExternalOutput")

    with tile.TileContext(nc) as tc:
        with tc.tile_pool(name="const", bufs=1) as cpool, \
             tc.tile_pool(name="xs", bufs=3) as xpool, \
             tc.tile_pool(name="tw", bufs=3) as twpool, \
             tc.tile_pool(name="gat", bufs=2) as gpool, \
             tc.tile_pool(name="oh", bufs=4) as ohpool, \
             tc.tile_pool(name="small", bufs=2) as spool, \
             tc.tile_pool(name="pers", bufs=1) as zpool, \
             tc.tile_pool(name="pacc", bufs=3, space="PSUM") as pacc, \
             tc.tile_pool(name="pmisc", bufs=2, space="PSUM") as pmisc, \
             tc.tile_pool(name="dram", bufs=1, space="DRAM") as dpool:

            def cload(dram_t, shape, dtype=F32):
                t = cpool.tile(shape, dtype, tag=dram_t.name)
                nc.sync.dma_start(t[:], dram_t[:])
                return t

            iota_sb = cload(iota_t, [P, P])
            ident_sb = cload(ident, [P, P])
            W1_sb = cload(W1, [DIN, DMID])
            W2a_sb = cload(W2a, [DMID, DOUT])
            W2b_sb = cload(W2b, [DATTR, DOUT])
            b1_sb = cload(b1col, [DMID, 1])
            b2_sb = cload(b2col, [DOUT, 1])
            Wmlo_sb = cload(Wmap_lo, [P, DEMB])
            Wmhi_sb = cload(Wmap_hi, [DH3 - P, DEMB])
            bmap_sb = cload(bmap, [1, DEMB])
            hsT_sb = cload(hsT_tiles, [P, KH])
            Wattn_sb = cload(Wattn_flat, [P, KH * DH3])
            ones8_sb = cload(ones8, [8, 1])
            ones1x_sb = cload(ones_1x128, [1, P])
            idx_sb = cload(idx_all, [P, idx_cols], I16)
            dstloc_sb = cload(dstloc_all, [P, G])
            w_sb = cload(w_all, [P, G])
            attT_sb = cload(attT, [DATTR, T * P])
            dinvc_sb = cload(dinv_col, [P, T])
            idxg_sb = cload(idx_grid, [P, 56])
            pcolb_sb = cload(pcol_base, [P, 1])
            base_sb = cload(base_11, [1, 1])
            invN_sb = cload(invN, [P, 1])

            # DRAM scratch
            g1_table = dpool.tile([ntab, DMID], F32)
            g2_shard = dpool.tile([shard, DOUT], F32)
            g2_table = dpool.tile([N, DOUT], F32)
            h3_dram = dpool.tile([shard, DH3P], F32)
            ag_in = dpool.tile([1, 224], F32)
            ag_out = dpool.tile([8, 224], F32)
            idx_dram = dpool.tile([1, 16], I16)

            # persistent SBUF
            zT = zpool.tile([P, T * P], F32, tag="zT")          # conv1 out ^T
            o2T = zpool.tile([P, T * P], F32, tag="o2T")        # conv2 out ^T
            scores = zpool.tile([P, 56], F32, tag="scores")
            neg_t = zpool.tile([P, 56], F32, tag="neg")
            topv = zpool.tile([1, K], F32, tag="topv")
            topi = zpool.tile([1, K], F32, tag="topi")
            nc.vector.memset(neg_t[:], NEG)

            # =================================================================
            # Phase A: g1 table (full, redundant per core):  g1 = xs @ W1
            # =================================================================
            XCH = 8
            tt = 0
            while tt < n_table_tiles:
                nt = min(XCH, n_table_tiles - tt)
                xs_sb = xpool.tile([DIN, XCH * P], F32, tag="xs")
                nc.sync.dma_start(xs_sb[:, :nt * P], xsT[:, tt * P:(tt + nt) * P])
                stage = twpool.tile([P, XCH * DMID], F32, tag="tw")
                for j in range(nt):
                    ps = pacc.tile([P, DMID], F32, tag="acc")
                    nc.tensor.matmul(ps[:], xs_sb[:, j * P:(j + 1) * P], W1_sb[:],
                                     start=True, stop=True)
                    nc.vector.tensor_copy(stage[:, j * DMID:(j + 1) * DMID], ps[:])
                nc.sync.dma_start(
                    g1_table[tt * P:(tt + nt) * P, :].rearrange(
                        "(n p) f -> p n f", n=nt),
                    stage[:, :nt * DMID])
                tt += nt

            # =================================================================
            # Conv aggregation pass (used for both convs)
            # =================================================================
            def conv_pass(table_of_half, accT, conv_id):
                # zero-init tiles whose pass-0 bucket is empty
                for t in range(T):
                    if bucket_groups[0, t] == 0:
                        nc.vector.memset(accT[:, t * P:(t + 1) * P], 0.0)
                psum_by_tile = {}
                for (h, g0, ng) in chunks:
                    gb = gpool.tile([P, CHUNK_G, DMID], F32, tag="gb")
                    nidx = ng * P
                    nc.gpsimd.dma_gather(
                        gb[:, :ng, :], table_of_half(h),
                        idx_sb[:, g0 * 8:(g0 + ng) * 8], nidx, nidx, DMID,
                        single_packet=False)
                    for j in range(ng):
                        g = g0 + j
                        t = int(group_tile[g])
                        first = g == int(gstart[h, t])
                        last = g == int(gstart[h, t] + bucket_groups[h, t] - 1)
                        if first:
                            psum_by_tile[t] = pacc.tile([P, P], F32, tag="acc",
                                                        name="acc_ps")
                        ps = psum_by_tile[t]
                        oh = ohpool.tile([P, P], F32, tag="oh")
                        nc.vector.tensor_scalar(
                            oh[:], iota_sb[:], dstloc_sb[:, g:g + 1],
                            w_sb[:, g:g + 1], OP.is_equal, OP.mult)
                        nc.tensor.matmul(ps[:], gb[:, j, :], oh[:],
                                         start=first, stop=last)
                        if last:
                            sl = accT[:, t * P:(t + 1) * P]
                            if h == 0:
                                nc.vector.tensor_copy(sl, ps[:])
                            else:
                                nc.vector.tensor_tensor(sl, sl, ps[:], op=OP.add)
                            del psum_by_tile[t]

            # =================================================================
            # Phase B: conv1 -> zT = relu(accT * dinv + b1)
            # =================================================================
            conv_pass(lambda h: g1_table[split:, :] if h else g1_table[:], zT, 1)
            for t in range(T):
                sl = zT[:, t * P:(t + 1) * P]
                nc.scalar.activation(sl, sl, AF.Relu, bias=b1_sb[:])

            # =================================================================
            # Phase C: g2 shard = dinv ⊙ (cat(z1, attr) @ W2); AllGather
            # =================================================================
            for t in range(T):
                ps = pacc.tile([P, DOUT], F32, tag="acc")
                nc.tensor.matmul(ps[:], zT[:, t * P:(t + 1) * P], W2a_sb[:],
                                 start=True, stop=False)
                nc.tensor.matmul(ps[:], attT_sb[:, t * P:(t + 1) * P], W2b_sb[:],
                                 start=False, stop=True)
                stage = twpool.tile([P, DOUT], F32, tag="g2s")
                nc.vector.tensor_scalar(stage[:], ps[:], dinvc_sb[:, t:t + 1],
                                        None, OP.mult)
                rows = min(P, shard - t * P)
                nc.sync.dma_start(g2_shard[t * P:t * P + rows, :],
                                  stage[:rows, :])
            nc.gpsimd.collective_compute(
                "AllGather", OP.bypass,
                replica_groups=[list(range(N_CORES))],
                ins=[g2_shard.opt()], outs=[g2_table.opt()])

            # =================================================================
            # Phase D: conv2 -> o2T = accT * dinv + b2;  h3 table to DRAM
            # =================================================================
            conv_pass(lambda h: g2_table[split:, :] if h else g2_table[:], o2T, 2)
            for t in range(T):
                sl = o2T[:, t * P:(t + 1) * P]
                nc.vector.tensor_scalar(sl, sl, b2_sb[:], None, OP.add)
            if last_rows < P:
                nc.vector.memset(o2T[:, (T - 1) * P + last_rows:T * P], 0.0)

            for t in range(T):
                pt = pmisc.tile([P, P], F32, tag="misc")
                nc.tensor.transpose(pt[:], o2T[:, t * P:(t + 1) * P], ident_sb[:])
                stage = twpool.tile([P, DH3P], F32, tag="h3s")
                nc.vector.tensor_copy(stage[:, :DOUT], pt[:])
                nc.sync.dma_start(stage[:, DOUT:DH3],
                                  att_tiles[:, t * DATTR:(t + 1) * DATTR])
                rows = min(P, shard - t * P)
                nc.sync.dma_start(h3_dram[t * P:t * P + rows, :],
                                  stage[:rows, :])

            if debug:
                nc.sync.dma_start(zT_out[:], zT[:])
                nc.sync.dma_start(g1t_out[:], g1_table[:256, :])
                nc.sync.dma_start(g2t_out[:], g2_table[:256, :])

            # =================================================================
            # Phase E: readout
            # =================================================================
            # q^T = (hidden_state @ Wattn)^T
            q_lo_p = pmisc.tile([P, 1], F32, tag="misc")
            for k in range(KH):
                nc.tensor.matmul(q_lo_p[:],
                                 Wattn_sb[:, k * DH3:k * DH3 + P],
                                 hsT_sb[:, k:k + 1],
                                 start=(k == 0), stop=(k == KH - 1))
            q_hi_p = pmisc.tile([DH3 - P, 1], F32, tag="misc")
            for k in range(KH):
                nc.tensor.matmul(q_hi_p[:],
                                 Wattn_sb[:, k * DH3 + P:(k + 1) * DH3],
                                 hsT_sb[:, k:k + 1],
                                 start=(k == 0), stop=(k == KH - 1))
            q_lo = spool.tile([P, 1], F32, tag="qlo")
            q_hi = spool.tile([DH3 - P, 1], F32, tag="qhi")
            nc.vector.tensor_copy(q_lo[:], q_lo_p[:])
            nc.vector.tensor_copy(q_hi[:], q_hi_p[:])

            nc.vector.memset(scores[:], NEG)
            for t in range(T):
                ps = pmisc.tile([P, 1], F32, tag="misc")
                nc.tensor.matmul(ps[:], o2T[:, t * P:(t + 1) * P], q_lo[:],
                                 start=True, stop=False)
                nc.tensor.matmul(ps[:], attT_sb[:, t * P:(t + 1) * P], q_hi[:],
                                 start=False, stop=True)
                rows = min(P, shard - t * P)
                nc.vector.tensor_copy(scores[:rows, t:t + 1], ps[:rows, :])

            if debug:
                nc.sync.dma_start(o2T_out[:], o2T[:])
                nc.sync.dma_start(sc_out[:], scores[:])

            # mean-pool partials
            mp_lo = spool.tile([P, 1], F32, tag="mplo")
            mp_hi = spool.tile([DH3 - P, 1], F32, tag="mphi")
            nc.vector.tensor_reduce(mp_lo[:], o2T[:, :T * P], axis=AX.X, op=OP.add)
            nc.vector.tensor_reduce(mp_hi[:], attT_sb[:, :T * P], axis=AX.X,
                                    op=OP.add)
            nc.vector.tensor_scalar(mp_lo[:], mp_lo[:], invN_sb[:], None, OP.mult)
            nc.vector.tensor_scalar(mp_hi[:], mp_hi[:], invN_sb[:DH3 - P, :],
                                    None, OP.mult)

            # local exact top-K extraction
            m8 = spool.tile([P, 8], F32, tag="m8")
            i8 = spool.tile([P, 8], U32, tag="i8")
            i8f = spool.tile([P, 1], F32, tag="i8f")
            gidx_col = spool.tile([P, 1], F32, tag="gidxc")
            colmax_r = spool.tile([1, P], F32, tag="cmr")
            gi_row = spool.tile([1, P], F32, tag="gir")
            sel_row = spool.tile([1, P], F32, tag="selr")
            mask_row = spool.tile([1, P], U32, tag="mkr")
            mask2d = spool.tile([P, 56], U32, tag="mk2")
            gmax = spool.tile([1, 1], F32, tag="gmax")
            gidx = spool.tile([1, 1], F32, tag="gidx")
            for r in range(K):
                nc.vector.max(m8[:], scores[:])
                nc.vector.max_index(i8[:], m8[:], scores[:])
                nc.vector.tensor_copy(i8f[:], i8[:, 0:1])
                nc.vector.tensor_scalar(i8f[:], i8f[:], 128.0, None, OP.mult)
                nc.vector.tensor_tensor(i8f[:], i8f[:], pcolb_sb[:], op=OP.add)
                pt = pmisc.tile([1, P], F32, tag="misc")
                nc.tensor.transpose(pt[:], m8[:, 0:1], ident_sb[:])
                nc.vector.tensor_copy(colmax_r[:], pt[:])
                pt2 = pmisc.tile([1, P], F32, tag="misc")
                nc.tensor.transpose(pt2[:], i8f[:], ident_sb[:])
                nc.vector.tensor_copy(gi_row[:], pt2[:])
                nc.vector.tensor_reduce(gmax[:], colmax_r[:], axis=AX.X, op=OP.max)
                nc.vector.tensor_scalar(mask_row[:], colmax_r[:], gmax[:],
                                        None, OP.is_equal)
                nc.vector.memset(sel_row[:], 1.0e30)
                nc.vector.copy_predicated(sel_row[:], mask_row[:], gi_row[:])
                nc.vector.tensor_reduce(gidx[:], sel_row[:], axis=AX.X, op=OP.min)
                nc.vector.tensor_copy(topv[:, r:r + 1], gmax[:])
                nc.vector.tensor_copy(topi[:, r:r + 1], gidx[:])
                pb = pmisc.tile([P, 1], F32, tag="misc")
                nc.tensor.matmul(pb[:], ones1x_sb[:], gidx[:], start=True, stop=True)
                nc.vector.tensor_copy(gidx_col[:], pb[:])
                nc.vector.tensor_scalar(mask2d[:], idxg_sb[:], gidx_col[:],
                                        None, OP.is_equal)
                nc.vector.copy_predicated(scores[:], mask2d[:], neg_t[:])

            if debug:
                nc.sync.dma_start(ti_out[:], topi[:])

            # small AllGather staging
            nc.sync.dma_start(ag_in[0:1, 0:P], mp_lo[:])
            nc.sync.dma_start(ag_in[0:1, P:DH3], mp_hi[:])
            nc.sync.dma_start(ag_in[0:1, DH3:DH3 + K], topv[:])
            nc.sync.dma_start(ag_in[0:1, DH3 + K:DH3 + 2 * K], topi[:])
            nc.gpsimd.collective_compute(
                "AllGather", OP.bypass,
                replica_groups=[list(range(N_CORES))],
                ins=[ag_in.opt()], outs=[ag_out.opt()])

            ag_sb = spool.tile([8, 224], F32, tag="agsb")
            nc.sync.dma_start(ag_sb[:], ag_out[:])

            # global mean -> emb
            gm_lo_p = pmisc.tile([P, 1], F32, tag="misc")
            nc.tensor.matmul(gm_lo_p[:], ag_sb[:, 0:P], ones8_sb[:],
                             start=True, stop=True)
            gm_hi_p = pmisc.tile([DH3 - P, 1], F32, tag="misc")
            nc.tensor.matmul(gm_hi_p[:], ag_sb[:, P:DH3], ones8_sb[:],
                             start=True, stop=True)
            gm_lo = spool.tile([P, 1], F32, tag="gmlo")
            gm_hi = spool.tile([DH3 - P, 1], F32, tag="gmhi")
            nc.vector.tensor_copy(gm_lo[:], gm_lo_p[:])
            nc.vector.tensor_copy(gm_hi[:], gm_hi_p[:])
            emb_p = pmisc.tile([1, DEMB], F32, tag="misc")
            nc.tensor.matmul(emb_p[:], gm_lo[:], Wmlo_sb[:], start=True, stop=False)
            nc.tensor.matmul(emb_p[:], gm_hi[:], Wmhi_sb[:], start=False, stop=True)
            emb_sb = spool.tile([1, DEMB], F32, tag="embsb")
            nc.vector.tensor_copy(emb_sb[:], emb_p[:])
            nc.vector.tensor_tensor(emb_sb[:], emb_sb[:], bmap_sb[:], op=OP.add)
            nc.sync.dma_start(emb_out[:], emb_sb[:])

            # global top-K merge over 8*K candidates
            cand_v = spool.tile([1, 8 * K], F32, tag="cv")
            cand_i = spool.tile([1, 8 * K], F32, tag="ci")
            nc.sync.dma_start(cand_v[:], ag_out[:, DH3:DH3 + K])
            nc.sync.dma_start(cand_i[:], ag_out[:, DH3 + K:DH3 + 2 * K])
            topi_g = spool.tile([1, K], F32, tag="tig")
            mrow = spool.tile([1, 8 * K], U32, tag="mrow")
            srow = spool.tile([1, 8 * K], F32, tag="srow")
            negrow = spool.tile([1, 8 * K], F32, tag="negrow")
            nc.vector.memset(negrow[:], NEG)
            for r in range(K):
                nc.vector.tensor_reduce(gmax[:], cand_v[:], axis=AX.X, op=OP.max)
                nc.vector.tensor_scalar(mrow[:], cand_v[:], gmax[:], None,
                                        OP.is_equal)
                nc.vector.memset(srow[:], 1.0e30)
                nc.vector.copy_predicated(srow[:], mrow[:], cand_i[:])
                nc.vector.tensor_reduce(gidx[:], srow[:], axis=AX.X, op=OP.min)
                nc.vector.tensor_copy(topi_g[:, r:r + 1], gidx[:])
                nc.vector.tensor_scalar(mrow[:], cand_i[:], gidx[:], None,
                                        OP.is_equal)
                nc.vector.copy_predicated(cand_v[:], mrow[:], negrow[:])

            if debug:
                nc.sync.dma_start(tig_out[:], topi_g[:])

            # fetch chosen rows owned by this shard
            loc_row = spool.tile([1, K], F32, tag="locr")
            clamp_row = spool.tile([1, K], F32, tag="clr")
            own_row = spool.tile([1, K], F32, tag="ownr")
            nc.vector.tensor_scalar(loc_row[:], topi_g[:], base_sb[:], None,
                                    OP.subtract)
            nc.vector.tensor_scalar(clamp_row[:], loc_row[:], 0.0,
                                    float(shard - 1), OP.max, OP.min)
            nc.vector.tensor_tensor(own_row[:], loc_row[:], clamp_row[:],
                                    op=OP.is_equal)
            pidx = pmisc.tile([K, 1], F32, tag="misc")
            nc.tensor.transpose(pidx[:], clamp_row[:], ident_sb[0:1, 0:1])
            pown = pmisc.tile([K, 1], F32, tag="misc")
            nc.tensor.transpose(pown[:], own_row[:], ident_sb[0:1, 0:1])
            idx16 = spool.tile([K, 1], I16, tag="idx16")
            nc.vector.tensor_copy(idx16[:], pidx[:])
            own128 = spool.tile([P, 1], F32, tag="own128")
            nc.vector.memset(own128[:], 0.0)
            nc.vector.tensor_copy(own128[:K, :], pown[:])
            nc.sync.dma_start(idx_dram[:], idx16[:])
            idx128 = spool.tile([P, 1], I16, tag="idx128")
            for grp in range(8):
                nc.sync.dma_start(idx128[grp * 16:(grp + 1) * 16, :], idx_dram[:])
            chose_b = gpool.tile([P, 1, DH3P], F32, tag="chose")
            nc.gpsimd.dma_gather(chose_b[:], h3_dram[:], idx128[:], K, K, DH3P)
            chose_s = spool.tile([P, DH3], F32, tag="choses")
            nc.vector.tensor_scalar(chose_s[:], chose_b[:, 0, :DH3],
                                    own128[:], None, OP.mult)
            nc.sync.dma_start(chose_out[:], chose_s[:K, :])

    nc.compile()
    return nc


# ----------------------------------------------------------------------------
# Entry point
# ----------------------------------------------------------------------------

_PER_CORE_KEYS = ("idx_all", "dstloc_all", "w_all", "att_tiles", "attT",
                  "dinv_col", "idx_grid", "pcol_base", "base_11", "invN")


def make_in_maps(meta, shared, per_core):
    in_maps = []
    for c in range(N_CORES):
        m = dict(shared)
        for k in _PER_CORE_KEYS:
            m[k] = per_core[c][k]
        in_maps.append(m)
    return in_maps


def assemble(meta, results):
    emb = results[0]["emb_out"]
    chose = np.zeros_like(results[0]["chose_out"])
    for r in results:
        chose += r["chose_out"]
    return np.concatenate([emb, chose.reshape(1, -1)], axis=1).astype(np.float32)


def kernel(**inputs) -> np.ndarray:
    meta, shared, per_core = _preprocess(
        inputs["x"], inputs["attributes"], inputs["edge_weight"],
        inputs["edge_index"], inputs["hidden_state"],
        inputs["W1"], inputs["b1"], inputs["W2"], inputs["b2"],
        inputs["Wmap"], inputs["bmap"], inputs["Wattn"])
    nc = build_program(meta)
    in_maps = make_in_maps(meta, shared, per_core)
    res = run_bass_kernel_spmd(nc, in_maps, core_ids=list(range(N_CORES)))
    return assemble(meta, res.results)
